# revision 1
# baseline (speedup 1.0000x reference)
"""DeepStitch Trainium2 Bass kernel (8-core, full-B replication).

Core 2b+par owns image b and descriptor half par.  Per core:
  conv A on its own row-half, n-PARTITIONED (psum [128n, 256c]; im2col as
  lhsT so positions land on partitions) -> fp16 feature rows streamed to
  DRAM fa_t[n, 256] -> resp row-sums (DVE segmented reduce, transposed
  spatial layout) -> per-block argmax in two stages (free-dim rows, then a
  diagonal-AP DRAM roundtrip to regroup blocks onto partitions, then cols)
  -> 128 descriptor indices -> one dma_gather(transpose=True) pulls the
  descriptors back c-partitioned [c, mb, k] -> conv B over the FULL image
  (replicated in the pair, c-partitioned) -> fp16 fb / fb^2 -> kNN scoring
  (4 fp16 matmuls per 512 positions), per-1024-chunk top-1 via DVE
  max/max_index on PSUM -> global argmin -> displacements -> single
  pairwise AllGather -> row/col MLPs (host reads even cores' out).

Conv bias rides as a 28th im2col row; conv-A weights are x4 and conv-B x2
so the gathered descriptors arrive pre-scaled (desc.fb - |fb|^2 =
4*(2ab - b^2), argmin-equivalent).  All conv matmuls in float32r (1 PE
cycle/row at free size >= 256); scoring in fp16 (verified index-exact vs
the fp32 reference).  im2col loads are single 28-descriptor DMAs from
host-prepped stride-2 tap planes [28, rows, 128].
"""

import sys

for _p in ("/opt/trn_rl_repo",):
    if _p not in sys.path:
        sys.path.insert(0, _p)

import numpy as np

import concourse.bacc as bacc
import concourse.bass as bass
import concourse.mybir as mybir
import concourse.tile as tile
import concourse.bass_utils as bass_utils
from concourse import library_config
from concourse.bass import AP
from contextlib import ExitStack

F32 = mybir.dt.float32
F32R = mybir.dt.float32r
F16 = mybir.dt.float16
I16 = mybir.dt.int16
I32 = mybir.dt.int32
U32 = mybir.dt.uint32
AF = mybir.ActivationFunctionType
ALU = mybir.AluOpType

B = 4
NCORES = 8
CIN = 3
COUT = 256
H = W = 128
NH = 8192           # half-image positions (conv-A share)
NF = 16384          # full-image positions (conv-B)

_DYS = {0: [0, 2], 1: [1]}


def _tap_order():
    taps = []
    for pr in (0, 1):
        for pc in (0, 1):
            for c in range(CIN):
                for dy in _DYS[pr]:
                    for dx in _DYS[pc]:
                        taps.append((c, dy, dx))
    assert len(taps) == 27
    return taps


TAPS = _tap_order()


def _prep_planes(x, r0, nrows):
    """[3,256,256] -> per-tap stride-2 planes [28, nrows, 128]; row 27 = ones
    (bias row, pairs with the bias row of the weight matrix)."""
    xp = np.zeros((CIN, 259, 259), dtype=np.float32)
    xp[:, 1:257, 1:257] = x
    out = np.empty((28, nrows, 128), dtype=np.float32)
    for t, (c, dy, dx) in enumerate(TAPS):
        sub = xp[c, dy::2, dx::2]
        out[t] = sub[r0 : r0 + nrows, :128]
    out[27] = 1.0
    return out


def _prep_w27a(Wconv, bconv):
    """conv-A weights: x4 (desc' = 4a so that desc'.fb' - |fb'|^2 =
    4*(2ab - b^2) with fb' = 2b)."""
    w = np.zeros((32, COUT), dtype=np.float32)
    for i, (c, dy, dx) in enumerate(TAPS):
        w[i] = 4.0 * Wconv[:, c, dy, dx]
    w[27] = 4.0 * bconv
    return w


def _prep_w27(Wconv, bconv):
    """[256,3,3,3] -> lhsT [32,256], x2 global scale; row 27 = 2*bias.
    The doubling makes gathered descriptors arrive pre-scaled (score =
    2a.b - |b|^2 computed as desc'.fb' - |fb'|^2 up to a x4 monotone
    factor)."""
    w = np.zeros((32, COUT), dtype=np.float32)
    for i, (c, dy, dx) in enumerate(TAPS):
        w[i] = 2.0 * Wconv[:, c, dy, dx]
    w[27] = 2.0 * bconv
    return w


def build_kernel(dbg=False):
    nc = bacc.Bacc("TRN2", target_bir_lowering=False, debug=False,
                   num_devices=NCORES)

    xa = nc.dram_tensor("xa", [28, 64, 128], F32R, kind="ExternalInput")
    xb = nc.dram_tensor("xb", [28, 128, 128], F32R, kind="ExternalInput")
    w27 = nc.dram_tensor("w27", [32, COUT], F32R, kind="ExternalInput")
    w27a = nc.dram_tensor("w27a", [32, COUT], F32R, kind="ExternalInput")
    negones = nc.dram_tensor("negones", [128, 128], F16, kind="ExternalInput")
    rowbl = nc.dram_tensor("rowbl", [128, 1], I32, kind="ExternalInput")
    colb = nc.dram_tensor("colb", [128, 1], I32, kind="ExternalInput")
    row64 = nc.dram_tensor("row64", [128, 1], I32, kind="ExternalInput")
    iota16 = nc.dram_tensor("iota16", [128, 16], F32, kind="ExternalInput")
    iota8 = nc.dram_tensor("iota8", [128, 8], F32, kind="ExternalInput")
    rpat = nc.dram_tensor("rpat", [128, 64], I32, kind="ExternalInput")
    w1 = nc.dram_tensor("w1", [2, 2, 128, 128], F32, kind="ExternalInput")
    b1 = nc.dram_tensor("b1", [128, 2], F32, kind="ExternalInput")
    w2 = nc.dram_tensor("w2", [128, 2], F32, kind="ExternalInput")
    b2 = nc.dram_tensor("b2", [1, 2], F32, kind="ExternalInput")
    out = nc.dram_tensor("out", [1, 2], F32, kind="ExternalOutput")
    scr = nc.dram_tensor("scr", [128], I16, kind="Internal")
    fa_t = nc.dram_tensor("fa_t", [NH, COUT], F16, kind="Internal")
    rvd = nc.dram_tensor("rvd", [2048], F32, kind="Internal")

    if dbg:
        na_dbg = nc.dram_tensor("na_dbg", [128, 1], I32, kind="ExternalOutput")
        desc_dbg = nc.dram_tensor("desc_dbg", [128, 2, 128], F32, kind="ExternalOutput")
        nb_dbg = nc.dram_tensor("nb_dbg", [128, 1], I32, kind="ExternalOutput")
        drow_dbg = nc.dram_tensor("drow_dbg", [128, 2], F32, kind="ExternalOutput")

    with tile.TileContext(nc) as tc, ExitStack() as ctx:
        const = ctx.enter_context(tc.tile_pool(name="const", bufs=1))
        small = ctx.enter_context(tc.tile_pool(name="small", bufs=1))
        fan_pool = ctx.enter_context(tc.tile_pool(name="fan", bufs=6))
        ima_pool = ctx.enter_context(tc.tile_pool(name="ima", bufs=2))
        imb_pool = ctx.enter_context(tc.tile_pool(name="imb", bufs=4))
        fbt_pool = ctx.enter_context(tc.tile_pool(name="fbt", bufs=3))
        dram = ctx.enter_context(tc.tile_pool(name="dram", bufs=1, space="DRAM"))
        cpsum = ctx.enter_context(tc.tile_pool(name="cpsum", bufs=2, space="PSUM"))
        spsum = ctx.enter_context(tc.tile_pool(name="spsum", bufs=2, space="PSUM"))

        def ld(name, shape, dt_, tensor, ap=None):
            t = const.tile(shape, dt_, tag=name)
            nc.sync.dma_start(t[:], ap if ap is not None else tensor.ap())
            return t

        w27_sb = ld("w27", [32, COUT], F32R, w27)
        w27a_sb = ld("w27a", [32, COUT], F32R, w27a)
        nones_sb = ld("nones", [128, 128], F16, negones)
        rowbl_sb = ld("rowbl", [128, 1], I32, rowbl)
        colb_sb = ld("colb", [128, 1], I32, colb)
        row64_sb = ld("row64", [128, 1], I32, row64)
        iota16_sb = ld("iota16", [128, 16], F32, iota16)
        iota8_sb = ld("iota8", [128, 8], F32, iota8)
        rpat_sb = ld("rpat", [128, 64], I32, rpat)
        w1_sb = ld("w1", [128, 2, 2, 128], F32, w1,
                   AP(tensor=w1, offset=0, ap=[[128, 128], [32768, 2], [16384, 2], [1, 128]]))
        b1_sb = ld("b1", [128, 2], F32, b1)
        w2_sb = ld("w2", [128, 2], F32, w2)
        b2_sb = ld("b2", [1, 2], F32, b2)


        # ================= Phase 1: conv A (n-partitioned only) ==========
        # psum [128n, 256c] chunks -> relu -> fp16 fan tiles -> fa_t rows;
        # resp[n] = sum_c fan (Pool reduce, transposed spatial layout:
        # partition = col, free = row).
        resp_nb = small.tile([128, 64], F32)
        for ha in range(2):
            im_a = ima_pool.tile([32, NH // 2], F32R, tag="im")
            nc.sync.dma_start(
                im_a[0:28, :],
                AP(tensor=xa, offset=ha * (NH // 2), ap=[[NH, 28], [1, NH // 2]]))
            for g in range(8):
                np_ = cpsum.tile([128, 1024], F32, tag="cps")
                for u in range(4):
                    nc.tensor.matmul(
                        np_[:, 256 * u : 256 * (u + 1)],
                        im_a[0:28, 128 * (4 * g + u) : 128 * (4 * g + u) + 128],
                        w27a_sb[0:28, :],
                        start=True, stop=True)
                fan = fan_pool.tile([128, 1024], F16, tag="fan")
                nc.scalar.activation(fan[:], np_[:], AF.Relu)
                nc.gpsimd.dma_start(
                    AP(tensor=fa_t, offset=(32 * ha + 4 * g) * 128 * COUT,
                       ap=[[COUT, 128], [128 * COUT, 4], [1, COUT]]),
                    fan[:].rearrange("p (u c) -> p u c", u=4))
                nc.vector.tensor_reduce(
                    resp_nb[:, 32 * ha + 4 * g : 32 * ha + 4 * g + 4],
                    fan[:].rearrange("p (u c) -> p u c", u=4),
                    axis=mybir.AxisListType.X, op=ALU.add)

        # ---- selection: packed-bits tree ----
        # resp_nb[p=col, r=row] >= 0, so fp32 bit-order == value order.
        # pack: clear 3 mantissa LSBs, or-in (7-u); one tree-max per band
        # then carries both the max and its row argmax, with reference
        # (u,v)-lexicographic tie order.
        rpk = small.tile([128, 64], F32)
        nc.vector.tensor_single_scalar(rpk[:].bitcast(I32),
                                       resp_nb[:].bitcast(I32), -8,
                                       ALU.bitwise_and)
        nc.vector.tensor_tensor(rpk[:].bitcast(I32), rpk[:].bitcast(I32),
                                rpat_sb[:], ALU.bitwise_or)
        rv = rpk[:].rearrange("p (gr r) -> p gr r", gr=8)
        t1 = small.tile([128, 8, 4], F32)
        t2 = small.tile([128, 8, 2], F32)
        rowwin = small.tile([128, 8], F32)
        nc.vector.tensor_tensor(t1[:], rv[:, :, 0:4], rv[:, :, 4:8], ALU.max)
        nc.vector.tensor_tensor(t2[:], t1[:, :, 0:2], t1[:, :, 2:4], ALU.max)
        nc.vector.tensor_tensor(rowwin[:].rearrange("p (gr o) -> p gr o", o=1),
                                t2[:, :, 0:1], t2[:, :, 1:2], ALU.max)
        nc.gpsimd.dma_start(
            AP(tensor=rvd, offset=0, ap=[[8, 128], [1, 8]]), rowwin[:])
        bpk = small.tile([128, 8], F32)
        nc.gpsimd.dma_start(
            bpk[:], AP(tensor=rvd, offset=0, ap=[[1, 8], [64, 16], [8, 8]]))

        vmax8 = small.tile([128, 8], F32)
        vidx8 = small.tile([128, 8], U32)
        tmpu = small.tile([128, 1], I32)
        tmpv = small.tile([128, 1], I32)
        rowa_l = small.tile([128, 1], I32)
        rowa_g = small.tile([128, 1], I32)
        cola_l = small.tile([128, 1], I32)
        na_l = small.tile([128, 1], I32)
        nc.vector.max(vmax8[:], bpk[:])
        nc.vector.max_index(vidx8[:], vmax8[:], bpk[:])
        # u = 7 - (winner_bits & 7); v = argmax position
        nc.vector.tensor_single_scalar(tmpu[:], vmax8[:, 0:1].bitcast(I32), 7,
                                       ALU.bitwise_and)
        nc.vector.tensor_single_scalar(tmpu[:], tmpu[:], -1, ALU.mult)
        nc.vector.tensor_single_scalar(tmpu[:], tmpu[:], 7, ALU.add)
        nc.vector.tensor_copy(tmpv[:].bitcast(U32), vidx8[:, 0:1])
        nc.vector.tensor_tensor(rowa_l[:], rowbl_sb[:], tmpu[:], ALU.add)
        nc.vector.tensor_tensor(cola_l[:], colb_sb[:], tmpv[:], ALU.add)
        nc.vector.tensor_single_scalar(na_l[:], rowa_l[:], 7, ALU.logical_shift_left)
        nc.vector.tensor_tensor(na_l[:], na_l[:], cola_l[:], ALU.add)
        nc.vector.tensor_tensor(rowa_g[:], rowa_l[:], row64_sb[:], ALU.add)
        if dbg:
            nc.sync.dma_start(na_dbg.ap(), na_l[:])

        # wrap na into gather idx layout via DRAM
        na_i16 = small.tile([128, 1], I16)
        nc.vector.tensor_copy(na_i16[:], na_l[:])
        nc.scalar.dma_start(AP(tensor=scr, offset=0, ap=[[1, 128]]), na_i16[:])
        idxw = small.tile([128, 8], I16)
        for g in range(8):
            eng = nc.scalar if g % 2 == 0 else nc.sync
            eng.dma_start(
                idxw[16 * g : 16 * (g + 1), :],
                AP(tensor=scr, offset=0, ap=[[1, 16], [16, 8]]))

        desc_i = small.tile([128, 2, 128], F16)   # [c, mb, k] (pre-scaled x2)

        # ============== Phase 2: conv B (full image) + scoring ============
        im_b = {}
        for hb in range(4):
            imt = imb_pool.tile([32, 4096], F32R, tag="imb")
            nc.sync.dma_start(
                imt[0:28, :],
                AP(tensor=xb, offset=hb * 4096, ap=[[2 * NH, 28], [1, 4096]]))
            im_b[hb] = imt

        tmax = small.tile([128, 16, 8], F32)
        tidx = small.tile([128, 16, 8], U32)

        def convb_group(s4):
            """2048 positions -> fb/fb2 [128, 2, 2048] fp16."""
            imt = im_b[s4 // 2]
            base = (s4 % 2) * 2048
            fb = fbt_pool.tile([128, 2, 2048], F16, tag="fb")
            fb2 = fbt_pool.tile([128, 2, 2048], F16, tag="fb2")
            for half in range(2):
                for mb in range(2):
                    ps = cpsum.tile([128, 1024], F32, tag="cps")
                    for q in range(2):
                        o = base + 1024 * half + 512 * q
                        nc.tensor.matmul(
                            ps[:, 512 * q : 512 * (q + 1)],
                            w27_sb[0:28, 128 * mb : 128 * (mb + 1)],
                            imt[0:28, o : o + 512],
                            start=True, stop=True)
                    dst = fb[:, mb, 1024 * half : 1024 * (half + 1)]
                    nc.scalar.activation(dst, ps[:], AF.Relu)
            for mb in range(2):
                nc.scalar.square(fb2[:, mb, :], fb[:, mb, :])
            return fb, fb2

        def score_chunk(c, fb, fb2):
            off = (c % 2) * 1024
            sp = spsum.tile([128, 1024], F32, tag="sp")
            for s in range(2):
                sl = slice(off + 512 * s, off + 512 * (s + 1))
                po = sp[:, 512 * s : 512 * (s + 1)]
                nc.tensor.matmul(po, desc_i[:, 0, :], fb[:, 0, sl], start=True, stop=False)
                nc.tensor.matmul(po, desc_i[:, 1, :], fb[:, 1, sl], start=False, stop=False)
                nc.tensor.matmul(po, nones_sb[:], fb2[:, 0, sl], start=False, stop=False)
                nc.tensor.matmul(po, nones_sb[:], fb2[:, 1, sl], start=False, stop=True)
            nc.vector.max(tmax[:, c, :], sp[:])
            nc.vector.max_index(tidx[:, c, :], tmax[:, c, :], sp[:])

        pend = {}
        for s4 in range(6):
            pend[s4] = convb_group(s4)

        # descriptor gather sits between conv-B groups so the Pool queue
        # (squares ahead of it) never head-blocks on unready inputs
        nc.gpsimd.dma_gather(
            desc_i[:], fa_t.ap(), idxw[:],
            num_idxs=128, num_idxs_reg=128, elem_size=COUT, transpose=True)
        if dbg:
            nc.gpsimd.dma_start(desc_dbg.ap(), desc_i[:])

        for s4 in (6, 7):
            pend[s4] = convb_group(s4)
        for s4 in range(8):
            fb, fb2 = pend.pop(s4)
            score_chunk(2 * s4, fb, fb2)
            score_chunk(2 * s4 + 1, fb, fb2)

        # ---- global winner over the 16 chunk-top-1s ----
        gmx8 = small.tile([128, 8], F32)
        gix8 = small.tile([128, 8], U32)
        cstar = small.tile([128, 1], U32)
        cstarf = small.tile([128, 1], F32)
        mask16 = small.tile([128, 16], F32)
        locf = small.tile([128, 1], F32)
        locu = small.tile([128, 1], U32)
        nb = small.tile([128, 1], I32)
        tview = tmax[:].rearrange("p a b -> p (a b)")[:, ::8]
        iview = tidx[:].rearrange("p a b -> p (a b)")[:, ::8]
        nc.vector.max(gmx8[:], tview)
        nc.vector.max_index(gix8[:], gmx8[:], tview)
        nc.vector.tensor_copy(cstar[:], gix8[:, 0:1])
        nc.vector.tensor_copy(cstarf[:], cstar[:])
        nc.vector.tensor_scalar(mask16[:], iota16_sb[:], cstarf[:], None, ALU.is_equal)
        nc.vector.tensor_tensor(mask16[:], mask16[:], iview, ALU.mult)
        nc.vector.tensor_reduce(locf[:], mask16[:], axis=mybir.AxisListType.X, op=ALU.add)
        nc.vector.tensor_copy(locu[:], locf[:])
        nc.vector.tensor_single_scalar(cstar[:], cstar[:], 10, ALU.logical_shift_left)
        nc.vector.tensor_tensor(nb[:].bitcast(U32), cstar[:], locu[:], ALU.add)
        if dbg:
            nc.sync.dma_start(nb_dbg.ap(), nb[:])

        # ---- displacements ----
        rowb_t = small.tile([128, 1], I32)
        colb_t = small.tile([128, 1], I32)
        di_t = small.tile([128, 1], I32)
        d_f = small.tile([128, 2], F32)
        nc.vector.tensor_single_scalar(rowb_t[:], nb[:], 7, ALU.logical_shift_right)
        nc.vector.tensor_single_scalar(colb_t[:], nb[:], 127, ALU.bitwise_and)
        nc.vector.tensor_tensor(di_t[:], rowb_t[:], rowa_g[:], ALU.subtract)
        nc.vector.tensor_copy(d_f[:, 0:1], di_t[:])
        nc.vector.tensor_tensor(di_t[:], cola_l[:], colb_t[:], ALU.subtract)
        nc.vector.tensor_copy(d_f[:, 1:2], di_t[:])
        if dbg:
            nc.sync.dma_start(drow_dbg.ap(), d_f[:])

        # ---- Exchange: AllGather displacement halves in the pair ----
        ex_in = dram.tile([128, 2], F32)
        ex_out = dram.tile([2, 128, 2], F32)
        nc.scalar.dma_start(ex_in[:], d_f[:])
        nc.gpsimd.collective_compute(
            "AllGather", ALU.bypass,
            replica_groups=[[0, 1], [2, 3], [4, 5], [6, 7]],
            ins=[ex_in.opt()], outs=[ex_out.opt()])
        d_all = small.tile([128, 2, 2], F32)   # [k, half, rc]
        nc.scalar.dma_start(d_all[:], ex_out[:].rearrange("r p c -> p r c"))

        # ---- MLPs ----
        out_sb = small.tile([1, 2], F32)
        hid = small.tile([128, 1], F32)
        for rc in range(2):
            hp = spsum.tile([128, 1024], F32, tag="sp")
            for half in range(2):
                nc.tensor.matmul(hp[:, 0:1], w1_sb[:, rc, half, :],
                                 d_all[:, half, rc : rc + 1],
                                 start=(half == 0), stop=(half == 1))
            nc.scalar.activation(hid[:], hp[:, 0:1], AF.Relu, bias=b1_sb[:, rc : rc + 1])
            op = spsum.tile([128, 1024], F32, tag="sp")
            nc.tensor.matmul(op[:1, 0:1], hid[:], w2_sb[:, rc : rc + 1],
                             start=True, stop=True)
            nc.scalar.activation(out_sb[:, rc : rc + 1], op[:1, 0:1], AF.Identity,
                                 bias=b2_sb[:, rc : rc + 1])
        nc.scalar.dma_start(out.ap(), out_sb[:])

    nc.compile()
    return nc


_NC_CACHE = {}


def _get_nc(dbg=False):
    if dbg not in _NC_CACHE:
        _NC_CACHE[dbg] = build_kernel(dbg=dbg)
    return _NC_CACHE[dbg]


def _host_inputs(inputs):
    xA = np.asarray(inputs["xA"], np.float32)
    xB = np.asarray(inputs["xB"], np.float32)
    Wc = np.asarray(inputs["Wconv"], dtype=np.float32)
    bc = np.asarray(inputs["bconv"], dtype=np.float32)
    w27 = _prep_w27(Wc, bc)
    w27a = _prep_w27a(Wc, bc)
    negones = -np.ones((128, 128), dtype=np.float16)
    p = np.arange(128)
    rowbl = (8 * (p // 16)).astype(np.int32).reshape(128, 1)
    colb_ = (8 * (p % 16)).astype(np.int32).reshape(128, 1)
    w1 = np.stack([
        np.asarray(inputs["W1r"], np.float32).reshape(2, 128, 128),
        np.asarray(inputs["W1c"], np.float32).reshape(2, 128, 128),
    ])
    b1 = np.stack([np.asarray(inputs["b1r"], np.float32), np.asarray(inputs["b1c"], np.float32)], 1)
    w2 = np.concatenate([np.asarray(inputs["W2r"], np.float32), np.asarray(inputs["W2c"], np.float32)], 1)
    b2 = np.stack([np.asarray(inputs["b2r"], np.float32), np.asarray(inputs["b2c"], np.float32)], 1).reshape(1, 2)
    iota16 = np.broadcast_to(np.arange(16, dtype=np.float32), (128, 16)).copy()

    iota8 = np.broadcast_to(np.arange(8, dtype=np.float32), (128, 8)).copy()
    rpat = np.broadcast_to(7 - (np.arange(64, dtype=np.int32) % 8), (128, 64)).copy()
    shared = dict(w27=w27, w27a=w27a, negones=negones,
                  rowbl=rowbl, colb=colb_, w1=w1, b1=b1, w2=w2, b2=b2,
                  iota16=iota16, iota8=iota8, rpat=rpat)
    in_maps = []
    for c in range(NCORES):
        b, par = c // 2, c % 2
        m = dict(shared)
        m["xa"] = _prep_planes(xA[b], 64 * par, 64)
        m["xb"] = _prep_planes(xB[b], 0, 128)
        m["row64"] = np.full((128, 1), 64 * par, np.int32)
        in_maps.append(m)
    return in_maps


def kernel(**inputs):
    nc = _get_nc(dbg=False)
    in_maps = _host_inputs(inputs)
    res = bass_utils.run_bass_kernel_spmd(nc, in_maps, core_ids=list(range(NCORES)))
    return np.concatenate([res.results[2 * b]["out"] for b in range(B)], axis=0)


def kernel_dbg(**inputs):
    nc = _get_nc(dbg=True)
    in_maps = _host_inputs(inputs)
    res = bass_utils.run_bass_kernel_spmd(nc, in_maps, core_ids=list(range(NCORES)))
    out = np.concatenate([res.results[2 * b]["out"] for b in range(B)], axis=0)
    return out, res.results

